# revision 5
# baseline (speedup 1.0000x reference)
"""JKNet-Maxpool GNN kernel for 8 Trainium2 NeuronCores — v2.

Strategy (graph/data parallel, dense-adjacency aggregation in fp8):
  - Shard dst nodes 8 ways (1250/core, padded to 1280 = 10 tiles of 128).
  - segment_sum over edges == A @ m with A[dst, src] the edge-count matrix.
    Counts are small ints -> EXACT in fp8_e4m3.  The whole per-core A^T
    (10240 x 1280 fp8 = 13 MB) stays RESIDENT in SBUF for all 7 layers.
  - Aggregation runs as DoubleRow fp8 matmuls (2 src tiles / instruction,
    2x PE rate): stationary = m pair chunks [128, 2, 128] fp8, moving =
    A^T pair chunks [128, 2, w] fp8, fp32 PSUM accumulation over 40 pairs.
  - m is quantized to fp8 with per-layer scales s_l = 224/max|m_l| computed
    host-side (passed as data, so the compiled program is input-independent).
  - Slice-major pipeline: dst dim processed in 512-col slices; after each
    slice: ReLU(+1/s_l scale+bias) -> bf16 x -> JK max -> GEMM of the node
    group -> quantize -> AllGather of that group.  The 3 AGs per layer are
    issued ~a slice-period before their consumers -> collectives hidden.
  - Final GCN layer (normalize=True) reuses the SAME A^T: the symmetric
    norm folds into per-src dinv scaling (pre-AG) and per-dst dinv scaling
    (post-transpose); self-loop term m/deg added from the local GEMM psum.
  - All DMA via HWDGE (sync + scalar rings); gpsimd only issues collectives.
"""

import numpy as np
import ml_dtypes

import concourse.bass as bass
import concourse.bacc as bacc
import concourse.mybir as mybir
import concourse.tile as tile
from concourse.bass_utils import run_bass_kernel_spmd
from concourse.masks import make_identity

BF16 = mybir.dt.bfloat16
F32 = mybir.dt.float32
FP8 = mybir.dt.float8e4
AF = mybir.ActivationFunctionType
ALU = mybir.AluOpType
AX = mybir.AxisListType
DR = mybir.MatmulPerfMode.DoubleRow

P = 128
C = 8
N_NODES = 10000
IN_FEATS = 512
U = 256
OUTP = 128          # padded final conv width (real 64)
OUT_REAL = 64
L = 6               # hidden GCN conv layers
NLOC_REAL = (N_NODES + C - 1) // C          # 1250
NT = (NLOC_REAL + P - 1) // P               # 10
NLOC = NT * P                               # 1280
KT = C * NT                                 # 80 src tiles
NFULL = KT * P                              # 10240
KT_IN = IN_FEATS // P                       # 4
KT_U = U // P                               # 2
SLICES = [(0, 512), (512, 512), (1024, 256)]
GROUPS = [(0, 4), (4, 4), (8, 2)]           # (first nt, n nts) per group
GN = [nn for _, nn in GROUPS]
# pair processing order: group-major, then shard, then pair-in-group
PAIRS = []
for _g, (_nt0, _nn) in enumerate(GROUPS):
    for _c in range(C):
        for _pp in range(_nn // 2):
            PAIRS.append((_g, _c, _pp))
NPAIR = len(PAIRS)                          # 40
AT_COLS = sum(NPAIR * 2 * w for _, w in SLICES)   # 102400

F8NP = ml_dtypes.float8_e4m3   # IEEE e4m3, max 240 == TRN FP8_EXP4
QMAX = 224.0                   # target max after scaling (240 with margin)


# ---------------------------------------------------------------- program
def build_nc() -> bass.Bass:
    nc = bacc.Bacc("TRN2", target_bir_lowering=False, num_devices=C)

    # ---- dram I/O (per-core contents supplied via in_maps)
    ATs_d = nc.dram_tensor("ATs", [P, AT_COLS], FP8, kind="ExternalInput")
    hT_d = nc.dram_tensor("hT", [KT_IN, P, NLOC], BF16, kind="ExternalInput")
    W0_d = nc.dram_tensor("W0", [KT_IN, P, U], BF16, kind="ExternalInput")
    Wh_d = nc.dram_tensor("Wh", [L - 1, KT_U, P, U], BF16, kind="ExternalInput")
    Wo_d = nc.dram_tensor("Wo", [KT_U, P, OUTP], BF16, kind="ExternalInput")
    biases_d = nc.dram_tensor("biases", [P, 2 * L], F32, kind="ExternalInput")
    # scales: cols 0..5 = s_l (quantize m_l), cols 6..11 = 1/s_l
    scales_d = nc.dram_tensor("scales", [P, 2 * L], F32, kind="ExternalInput")
    # fincons: cols 0..9 dinv*s_fin | 10..19 dinv/s_fin | 20..29 1/deg
    fin_d = nc.dram_tensor("fincons", [P, 3 * NT], F32, kind="ExternalInput")
    bob_d = nc.dram_tensor("bob", [P, OUT_REAL], F32, kind="ExternalInput")
    out_d = nc.dram_tensor("out", [NLOC, OUT_REAL], F32, kind="ExternalOutput")

    with tile.TileContext(nc) as tc:
        with (
            tc.tile_pool(name="const", bufs=1) as const_p,
            tc.tile_pool(name="wpool", bufs=1) as w_p,
            tc.tile_pool(name="atres", bufs=1) as at_p,
            tc.tile_pool(name="x", bufs=6) as x_p,
            tc.tile_pool(name="jk", bufs=1) as jk_p,
            tc.tile_pool(name="mg01", bufs=2) as mg01_p,
            tc.tile_pool(name="mg2", bufs=2) as mg2_p,
            tc.tile_pool(name="mfin", bufs=1) as mfin_p,
            tc.tile_pool(name="stg", bufs=3) as stg_p,
            tc.tile_pool(name="selfbo", bufs=NT) as sb_p,
            tc.tile_pool(name="aggF", bufs=1) as af_p,
            tc.tile_pool(name="small", bufs=12) as sm_p,
            tc.tile_pool(name="psagg", bufs=4, space="PSUM") as psagg_p,
            tc.tile_pool(name="psmm", bufs=2, space="PSUM") as psmm_p,
            tc.tile_pool(name="pstr", bufs=2, space="PSUM") as pstr_p,
            tc.tile_pool(name="dram", bufs=1, space="DRAM") as dram_p,
        ):
            sdma = nc.sync.dma_start      # HWDGE ring 1: bulk loads
            adma = nc.scalar.dma_start    # HWDGE ring 2: small/latency writes

            # ---- dummy collective: absorbs the one-time CC-init barrier
            # (~40us) before any real AllGather needs the ring
            dmy_l = dram_p.tile([P, 4], FP8, name="dmy_loc")
            dmy_f = dram_p.tile([C, P, 4], FP8, name="dmy_full",
                                addr_space="Shared")
            nc.gpsimd.collective_compute(
                "AllGather", ALU.bypass, replica_groups=[list(range(C))],
                ins=[dmy_l.opt()], outs=[dmy_f.opt()],
            )

            # ---- constants
            biases = const_p.tile([P, 2 * L], F32, name="biases_sb")
            sdma(out=biases[:], in_=biases_d[:])
            scales = const_p.tile([P, 2 * L], F32, name="scales_sb")
            sdma(out=scales[:], in_=scales_d[:])
            fincons = const_p.tile([P, 3 * NT], F32, name="fincons_sb")
            sdma(out=fincons[:], in_=fin_d[:])
            bob = const_p.tile([P, OUT_REAL], F32, name="bob_sb")
            sdma(out=bob[:], in_=bob_d[:])
            ident = const_p.tile([P, P], F32, name="ident")
            make_identity(nc, ident[:])

            # ---- weights (bf16, resident)
            w0_sb = []
            for k in range(KT_IN):
                t = w_p.tile([P, U], BF16, name=f"w0_{k}")
                sdma(out=t[:], in_=W0_d[k])
                w0_sb.append(t)
            wh_sb = []
            for l in range(L - 1):
                row = []
                for k in range(KT_U):
                    t = w_p.tile([P, U], BF16, name=f"wh_{l}_{k}")
                    sdma(out=t[:], in_=Wh_d[l, k])
                    row.append(t)
                wh_sb.append(row)
            wo_sb = []
            for k in range(KT_U):
                t = w_p.tile([P, OUTP], BF16, name=f"wo_{k}")
                sdma(out=t[:], in_=Wo_d[k])
                wo_sb.append(t)

            # ---- h^T (layer-0 GEMM stationary), bf16
            hT_sb = []
            for k in range(KT_IN):
                t = x_p.tile([P, NLOC], BF16, tag="x", name=f"ht_{k}")
                sdma(out=t[:], in_=hT_d[k])
                hT_sb.append(t)

            # ---- A^T resident in SBUF, fp8, slice-major pair layout
            at_sb = []
            off = 0
            for s, (_, w) in enumerate(SLICES):
                ncols = NPAIR * 2 * w
                t = at_p.tile([P, NPAIR, 2, w], FP8, name=f"at_s{s}")
                sdma(out=t[:], in_=ATs_d[:, off:off + ncols])
                at_sb.append(t)
                off += ncols

            # ---- JK running max (bf16)
            jk = [jk_p.tile([P, NLOC], BF16, name=f"jk_{ft}") for ft in range(KT_U)]

            # ---- collective buffers per (conv, group)
            loc_d, full_d = {}, {}
            for l in range(L + 1):
                width = U if l < L else OUTP
                for g in range(3):
                    loc_d[(l, g)] = dram_p.tile(
                        [P, GN[g] * width], FP8, name=f"loc_{l}_{g}")
                    full_d[(l, g)] = dram_p.tile(
                        [C, P, GN[g] * width], FP8, name=f"full_{l}_{g}",
                        addr_space="Shared")

            rg = [list(range(C))]

            def gemm_group(l, g, xt_tiles, w_tiles):
                """GEMM node-group g of conv l's messages, quantize to fp8,
                write to DRAM, AllGather.  l==L is the final conv (from jk)."""
                nt0, nn = GROUPS[g]
                width = U if l < L else OUTP
                stage = stg_p.tile([P, 4 * U], FP8, tag="stg",
                                   name=f"stg_{l}_{g}")
                for i in range(nn):
                    nt = nt0 + i
                    ps = psmm_p.tile([P, U], F32, tag="mm", name=f"mm_{l}_{g}_{i}")
                    kt = len(xt_tiles)
                    for k in range(kt):
                        nc.tensor.matmul(
                            ps[:, :width],
                            lhsT=xt_tiles[k][:, nt * P:(nt + 1) * P],
                            rhs=w_tiles[k][:, :width],
                            start=(k == 0), stop=(k == kt - 1),
                        )
                    if l < L:
                        qs = scales[:, l:l + 1]
                    else:
                        qs = fincons[:, nt:nt + 1]  # dinv * s_fin (per node)
                    nc.vector.tensor_scalar_mul(
                        stage[:, i * width:(i + 1) * width], ps[:, :width], qs)
                    if l == L:
                        # self-loop term for the final conv: m/deg + bo
                        t1 = sm_p.tile([P, OUT_REAL], F32, tag="sm",
                                       name=f"st1_{nt}")
                        nc.vector.tensor_scalar_mul(
                            t1[:], ps[:, :OUT_REAL], fincons[:, 2 * NT + nt:2 * NT + nt + 1])
                        t2 = sb_p.tile([P, OUT_REAL], F32, tag="selfbo",
                                       name=f"selfbo_{nt}")
                        nc.vector.tensor_tensor(
                            out=t2[:], in0=t1[:], in1=bob[:], op=ALU.add)
                        selfbo[nt] = t2
                adma(out=loc_d[(l, g)][:], in_=stage[:, :nn * width])
                nc.gpsimd.collective_compute(
                    "AllGather", ALU.bypass, replica_groups=rg,
                    ins=[loc_d[(l, g)].opt()],
                    outs=[full_d[(l, g)].opt()],
                )

            def load_m(l):
                """Load the 3 gathered message groups of conv l into SBUF.
                Alternate HWDGE rings so the loads don't serialize."""
                width = U if l < L else OUTP
                tiles = []
                for g in range(3):
                    nn = GN[g]
                    if l == L:
                        t = mfin_p.tile([P, C, nn, OUTP], FP8, tag=f"mf{g}",
                                        name=f"mfin_{g}")
                    elif g < 2:
                        t = mg01_p.tile([P, C, nn, U], FP8, tag=f"m{g}",
                                        name=f"m_{l}_{g}")
                    else:
                        t = mg2_p.tile([P, C, nn, U], FP8, tag="m2",
                                       name=f"m_{l}_{g}")
                    dma = adma if g == 1 else sdma
                    dma(out=t[:], in_=full_d[(l, g)][:].rearrange("c p x -> p c x"))
                    tiles.append(t)
                return tiles

            selfbo = [None] * NT

            # ================= conv 0 messages =================
            for g in range(3):
                gemm_group(0, g, hT_sb, w0_sb)

            # ================= conv layers =================
            xt = None
            for l in range(L):
                m_tiles = load_m(l)
                xt_new = [
                    x_p.tile([P, NLOC], BF16, tag="x", name=f"x{l + 1}_{ft}")
                    for ft in range(KT_U)
                ]
                for s, (off, w) in enumerate(SLICES):
                    pss = [
                        psagg_p.tile([P, 512], F32, tag="agg",
                                     name=f"agg_{l}_{s}_{ft}")
                        for ft in range(KT_U)
                    ]
                    for j, (g, c, pp) in enumerate(PAIRS):
                        rhs = at_sb[s][:, j]
                        for ft in range(KT_U):
                            nc.tensor.matmul(
                                pss[ft][:, :w],
                                lhsT=m_tiles[g][:, c, pp * 2:pp * 2 + 2,
                                                ft * P:(ft + 1) * P],
                                rhs=rhs,
                                start=(j == 0), stop=(j == NPAIR - 1),
                                perf_mode=DR,
                            )
                    # evacuate slice: x = relu(psum/s_l + b), jk = max(jk, x)
                    for ft in range(KT_U):
                        nc.scalar.activation(
                            xt_new[ft][:, off:off + w], pss[ft][:, :w], AF.Relu,
                            bias=biases[:, 2 * l + ft:2 * l + ft + 1],
                            scale=scales[:, L + l:L + l + 1],
                        )
                        if l == 0:
                            nc.vector.tensor_copy(
                                out=jk[ft][:, off:off + w],
                                in_=xt_new[ft][:, off:off + w])
                        else:
                            nc.vector.tensor_tensor(
                                out=jk[ft][:, off:off + w],
                                in0=jk[ft][:, off:off + w],
                                in1=xt_new[ft][:, off:off + w], op=ALU.max)
                    # produce next conv's messages for this node group + AG
                    if l < L - 1:
                        gemm_group(l + 1, s, xt_new, wh_sb[l])
                    else:
                        gemm_group(L, s, jk, wo_sb)
                xt = xt_new

            # ================= final conv aggregation =================
            m_tiles = load_m(L)
            aggF = af_p.tile([P, NLOC], F32, name="aggF")
            for s, (off, w) in enumerate(SLICES):
                ps = psagg_p.tile([P, 512], F32, tag="agg", name=f"aggf_{s}")
                for j, (g, c, pp) in enumerate(PAIRS):
                    nc.tensor.matmul(
                        ps[:, :w],
                        lhsT=m_tiles[g][:, c, pp * 2:pp * 2 + 2, 0:OUTP],
                        rhs=at_sb[s][:, j],
                        start=(j == 0), stop=(j == NPAIR - 1),
                        perf_mode=DR,
                    )
                nc.vector.tensor_copy(out=aggF[:, off:off + w], in_=ps[:, :w])

            # ================= normalize + self loop + log_softmax ==========
            # stage-major batching: engines pipeline across node tiles and
            # the scalar engine loads each activation table exactly once
            z3s, lsums = [], []
            for nt in range(NT):
                ps_t = pstr_p.tile([P, P], F32, tag="tr", name=f"tr_{nt}")
                nc.tensor.transpose(
                    out=ps_t[:], in_=aggF[:, nt * P:(nt + 1) * P],
                    identity=ident[:])
                z = sm_p.tile([P, OUT_REAL], F32, tag="sm", name=f"z_{nt}")
                nc.vector.tensor_scalar_mul(
                    z[:], ps_t[:, :OUT_REAL], fincons[:, NT + nt:NT + nt + 1])
                z2 = sm_p.tile([P, OUT_REAL], F32, tag="sm", name=f"z2_{nt}")
                nc.vector.tensor_tensor(
                    out=z2[:], in0=z[:], in1=selfbo[nt][:], op=ALU.add)
                rmax = sm_p.tile([P, 1], F32, tag="r1", name=f"rmax_{nt}")
                nc.vector.reduce_max(rmax[:], z2[:], axis=AX.X)
                z3 = sb_p.tile([P, OUT_REAL], F32, tag="z3", name=f"z3_{nt}")
                nc.vector.tensor_scalar_sub(z3[:], z2[:], rmax[:])
                z3s.append(z3)
            ezs = []
            for nt in range(NT):
                ez = sm_p.tile([P, OUT_REAL], F32, tag="sm", name=f"ez_{nt}")
                nc.scalar.activation(ez[:], z3s[nt][:], AF.Exp)
                ezs.append(ez)
            ssums = []
            for nt in range(NT):
                ssum = sm_p.tile([P, 1], F32, tag="r1", name=f"ssum_{nt}")
                nc.vector.reduce_sum(ssum[:], ezs[nt][:], axis=AX.X)
                ssums.append(ssum)
            for nt in range(NT):
                lsum = sm_p.tile([P, 1], F32, tag="r1", name=f"lsum_{nt}")
                nc.scalar.activation(lsum[:], ssums[nt][:], AF.Ln)
                lsums.append(lsum)
            for nt in range(NT):
                o = sm_p.tile([P, OUT_REAL], F32, tag="sm", name=f"o_{nt}")
                nc.vector.tensor_scalar_sub(o[:], z3s[nt][:], lsums[nt][:])
                adma(out=out_d[nt * P:(nt + 1) * P, :], in_=o[:])

    nc.compile()
    return nc


# ---------------------------------------------------------------- host prep
def _forward_scales(h, edge_index, W0, b0, Wh, bh, Wo, bo, deg, dinv):
    """Cheap fp32 forward (sparse) to get per-layer max|m| for fp8 scaling."""
    import scipy.sparse as sp
    src = np.asarray(edge_index[0], np.int64)
    dst = np.asarray(edge_index[1], np.int64)
    A = sp.csr_matrix(
        (np.ones(len(src), np.float32), (dst, src)), shape=(N_NODES, N_NODES))
    x = np.asarray(h, np.float32)
    smax = []
    outs = []
    for l in range(L):
        W = np.asarray(W0 if l == 0 else Wh[l - 1], np.float32)
        b = np.asarray(b0 if l == 0 else bh[l - 1], np.float32)
        m = x @ W
        smax.append(np.abs(m).max())
        x = np.maximum(A @ m + b, 0.0)
        outs.append(x)
    xj = np.max(np.stack(outs), 0)
    mo = xj @ np.asarray(Wo, np.float32)
    smax.append(np.abs(mo * dinv[:, None]).max())
    return smax


def host_prep(h, edge_index, W0, b0, Wh, bh, Wo, bo):
    bf = ml_dtypes.bfloat16
    src = np.asarray(edge_index[0], np.int64)
    dst = np.asarray(edge_index[1], np.int64)

    deg = np.zeros(N_NODES, np.float32)
    np.add.at(deg, dst, 1.0)
    deg += 1.0
    dinv = (deg ** -0.5).astype(np.float32)

    smax = _forward_scales(h, edge_index, W0, b0, Wh, bh, Wo, bo, deg, dinv)
    s_hid = [QMAX / max(v, 1e-30) for v in smax[:L]]
    s_fin = QMAX / max(smax[L], 1e-30)

    # padded global src index: core r, local i -> r*NLOC + i
    psrc = (src // NLOC_REAL) * NLOC + (src % NLOC_REAL)

    # shared (node-independent) tensors
    W0_a = np.asarray(W0, np.float32).astype(bf).reshape(KT_IN, P, U)
    Wh_a = np.asarray(Wh, np.float32).astype(bf).reshape(L - 1, KT_U, P, U)
    Wo_pad = np.zeros((U, OUTP), np.float32)
    Wo_pad[:, :OUT_REAL] = np.asarray(Wo, np.float32)
    Wo_a = Wo_pad.astype(bf).reshape(KT_U, P, OUTP)
    biases = np.zeros((P, 2 * L), np.float32)
    for l in range(L):
        b = np.asarray(b0 if l == 0 else bh[l - 1], np.float32)
        for ft in range(KT_U):
            biases[:, 2 * l + ft] = b[ft * P:(ft + 1) * P]
    scales = np.zeros((P, 2 * L), np.float32)
    for l in range(L):
        scales[:, l] = s_hid[l]
        scales[:, L + l] = 1.0 / s_hid[l]
    bob = np.broadcast_to(
        np.asarray(bo, np.float32)[None, :OUT_REAL], (P, OUT_REAL)).copy()

    in_maps = []
    for c in range(C):
        lo, hi = c * NLOC_REAL, min((c + 1) * NLOC_REAL, N_NODES)
        sel = (dst >= lo) & (dst < hi)
        s_c = psrc[sel]
        d_c = dst[sel] - lo

        cnt = np.bincount(s_c * NLOC + d_c, minlength=NFULL * NLOC)
        A3 = cnt.astype(np.float32).reshape(KT, P, NLOC)
        assert cnt.max() <= 16, "edge multiplicity too large for exact fp8"

        blocks = []
        for s, (off, w) in enumerate(SLICES):
            for (g, cc, pp) in PAIRS:
                nt0 = GROUPS[g][0] + 2 * pp
                t0 = cc * NT + nt0
                blocks.append(A3[t0, :, off:off + w])
                blocks.append(A3[t0 + 1, :, off:off + w])
        ATs = np.concatenate(blocks, axis=1).astype(F8NP)

        hT = np.zeros((IN_FEATS, NLOC), np.float32)
        hT[:, :hi - lo] = np.asarray(h[lo:hi], np.float32).T
        hT = hT.astype(bf).reshape(KT_IN, P, NLOC)

        dinv_l = np.ones(NLOC, np.float32)
        deg_l = np.ones(NLOC, np.float32)
        dinv_l[:hi - lo] = dinv[lo:hi]
        deg_l[:hi - lo] = deg[lo:hi]
        fincons = np.zeros((P, 3 * NT), np.float32)
        for nt in range(NT):
            sl = slice(nt * P, (nt + 1) * P)
            fincons[:, nt] = dinv_l[sl] * s_fin
            fincons[:, NT + nt] = dinv_l[sl] / s_fin
            fincons[:, 2 * NT + nt] = 1.0 / deg_l[sl]

        in_maps.append({
            "ATs": ATs,
            "hT": hT.copy(),
            "W0": W0_a.copy(),
            "Wh": Wh_a.copy(),
            "Wo": Wo_a.copy(),
            "biases": biases.copy(),
            "scales": scales.copy(),
            "fincons": fincons,
            "bob": bob.copy(),
        })
    return in_maps


_CACHE = {}


def _get_nc():
    if "nc" not in _CACHE:
        _CACHE["nc"] = build_nc()
    return _CACHE["nc"]


def kernel(h, edge_index, W0, b0, Wh, bh, Wo, bo, _trace=False, _trace_kwargs=None):
    nc = _get_nc()
    in_maps = host_prep(h, edge_index, W0, b0, Wh, bh, Wo, bo)
    res = run_bass_kernel_spmd(
        nc, in_maps, list(range(C)),
        trace=_trace, **(_trace_kwargs or {}),
    )
    outs = [np.asarray(res.results[c]["out"])[:NLOC_REAL] for c in range(C)]
    full = np.concatenate(outs, axis=0)[:N_NODES].astype(np.float32)
    if _trace:
        return full, res
    return full


# revision 10
# speedup vs baseline: 1.1355x; 1.1355x over previous
"""JKNet-Maxpool GNN kernel for 8 Trainium2 NeuronCores — v2.

Strategy (graph/data parallel, dense-adjacency aggregation in fp8):
  - Shard dst nodes 8 ways (1250/core, padded to 1280 = 10 tiles of 128).
  - segment_sum over edges == A @ m with A[dst, src] the edge-count matrix.
    Counts are small ints -> EXACT in fp8_e4m3.  The whole per-core A^T
    (10240 x 1280 fp8 = 13 MB) stays RESIDENT in SBUF for all 7 layers.
  - Aggregation runs as DoubleRow fp8 matmuls (2 src tiles / instruction,
    2x PE rate): stationary = m pair chunks [128, 2, 128] fp8, moving =
    A^T pair chunks [128, 2, w] fp8, fp32 PSUM accumulation over 40 pairs.
  - m is quantized to fp8 with per-layer scales s_l = 224/max|m_l| computed
    host-side (passed as data, so the compiled program is input-independent).
  - Slice-major pipeline: dst dim processed in 512-col slices; after each
    slice: ReLU(+1/s_l scale+bias) -> bf16 x -> JK max -> GEMM of the node
    group -> quantize -> AllGather of that group.  The 3 AGs per layer are
    issued ~a slice-period before their consumers -> collectives hidden.
  - Final GCN layer (normalize=True) reuses the SAME A^T: the symmetric
    norm folds into per-src dinv scaling (pre-AG) and per-dst dinv scaling
    (post-transpose); self-loop term m/deg added from the local GEMM psum.
  - All DMA via HWDGE (sync + scalar rings); gpsimd only issues collectives.
"""

import numpy as np
import ml_dtypes

import concourse.bass as bass
import concourse.bacc as bacc
import concourse.mybir as mybir
import concourse.tile as tile
from concourse.bass_utils import run_bass_kernel_spmd
from concourse.masks import make_identity

BF16 = mybir.dt.bfloat16
F32 = mybir.dt.float32
FP8 = mybir.dt.float8e4
AF = mybir.ActivationFunctionType
ALU = mybir.AluOpType
AX = mybir.AxisListType
DR = mybir.MatmulPerfMode.DoubleRow

P = 128
C = 8
N_NODES = 10000
IN_FEATS = 512
U = 256
KP_IN = IN_FEATS // 256     # 2 fp8 DoubleRow k-pairs for conv0
OUTP = 128          # padded final conv width (real 64)
OUT_REAL = 64
L = 6               # hidden GCN conv layers
NLOC_REAL = (N_NODES + C - 1) // C          # 1250
NT = (NLOC_REAL + P - 1) // P               # 10
NLOC = NT * P                               # 1280
KT = C * NT                                 # 80 src tiles
NFULL = KT * P                              # 10240
KT_IN = IN_FEATS // P                       # 4
KT_U = U // P                               # 2
SLICES = [(0, 512), (512, 512), (1024, 256)]
GROUPS = [(0, 4), (4, 4), (8, 2)]           # (first nt, n nts) per group
GN = [nn for _, nn in GROUPS]
# pair processing order: group-major, then shard, then pair-in-group
PAIRS = []
for _g, (_nt0, _nn) in enumerate(GROUPS):
    for _c in range(C):
        for _pp in range(_nn // 2):
            PAIRS.append((_g, _c, _pp))
NPAIR = len(PAIRS)                          # 40
AT_COLS = sum(NPAIR * 2 * w for _, w in SLICES)   # 102400

F8NP = ml_dtypes.float8_e4m3   # IEEE e4m3, max 240 == TRN FP8_EXP4
QMAX = 224.0                   # target max after scaling (240 with margin)


# ---------------------------------------------------------------- program
def build_nc() -> bass.Bass:
    nc = bacc.Bacc("TRN2", target_bir_lowering=False, num_devices=C)

    # ---- dram I/O (per-core contents supplied via in_maps)
    ATs_d = nc.dram_tensor("ATs", [P, AT_COLS], FP8, kind="ExternalInput")
    hT_d = nc.dram_tensor("hT", [KT_IN, P, NLOC], BF16, kind="ExternalInput")
    W0_d = nc.dram_tensor("W0", [KT_IN, P, U], BF16, kind="ExternalInput")
    Wh_d = nc.dram_tensor("Wh", [L - 1, KT_U, P, U], BF16, kind="ExternalInput")
    Wo_d = nc.dram_tensor("Wo", [KT_U, P, OUTP], BF16, kind="ExternalInput")
    biases_d = nc.dram_tensor("biases", [P, 2 * L], F32, kind="ExternalInput")
    # scales: cols 0..5 = s_l (quantize m_l), cols 6..11 = 1/s_l
    scales_d = nc.dram_tensor("scales", [P, 2 * L], F32, kind="ExternalInput")
    # fincons: cols 0..9 dinv*s_fin | 10..19 dinv/s_fin | 20..29 1/deg
    fin_d = nc.dram_tensor("fincons", [P, 3 * NT], F32, kind="ExternalInput")
    bob_d = nc.dram_tensor("bob", [P, OUT_REAL], F32, kind="ExternalInput")
    out_d = nc.dram_tensor("out", [NLOC, OUT_REAL], F32, kind="ExternalOutput")

    with tile.TileContext(nc) as tc:
        with (
            tc.tile_pool(name="const", bufs=1) as const_p,
            tc.tile_pool(name="wpool", bufs=1) as w_p,
            tc.tile_pool(name="atres", bufs=1) as at_p,
            tc.tile_pool(name="x", bufs=6) as x_p,
            tc.tile_pool(name="jk", bufs=1) as jk_p,
            tc.tile_pool(name="mg01", bufs=2) as mg01_p,
            tc.tile_pool(name="mg2", bufs=2) as mg2_p,
            tc.tile_pool(name="mfin", bufs=1) as mfin_p,
            tc.tile_pool(name="stg", bufs=3) as stg_p,
            tc.tile_pool(name="selfbo", bufs=NT) as sb_p,
            tc.tile_pool(name="aggF", bufs=1) as af_p,
            tc.tile_pool(name="small", bufs=12) as sm_p,
            tc.tile_pool(name="psagg", bufs=4, space="PSUM") as psagg_p,
            tc.tile_pool(name="psmm", bufs=2, space="PSUM") as psmm_p,
            tc.tile_pool(name="pstr", bufs=2, space="PSUM") as pstr_p,
            tc.tile_pool(name="dram", bufs=1, space="DRAM") as dram_p,
        ):
            sdma = nc.sync.dma_start      # HWDGE ring 1: bulk loads
            adma = nc.scalar.dma_start    # HWDGE ring 2: small/latency writes

            # ---- constants
            biases = const_p.tile([P, 2 * L], F32, name="biases_sb")
            sdma(out=biases[:], in_=biases_d[:])
            scales = const_p.tile([P, 2 * L], F32, name="scales_sb")
            sdma(out=scales[:], in_=scales_d[:])
            fincons = const_p.tile([P, 3 * NT], F32, name="fincons_sb")
            sdma(out=fincons[:], in_=fin_d[:])
            bob = const_p.tile([P, OUT_REAL], F32, name="bob_sb")
            sdma(out=bob[:], in_=bob_d[:])
            ident = const_p.tile([P, P], F32, name="ident")
            make_identity(nc, ident[:])

            # ---- weights (bf16, resident)
            w0_sb = []
            for k in range(KT_IN):
                t = w_p.tile([P, U], BF16, name=f"w0_{k}")
                sdma(out=t[:], in_=W0_d[k])
                w0_sb.append(t)
            wh_sb = []
            for l in range(L - 1):
                row = []
                for k in range(KT_U):
                    t = w_p.tile([P, U], BF16, name=f"wh_{l}_{k}")
                    sdma(out=t[:], in_=Wh_d[l, k])
                    row.append(t)
                wh_sb.append(row)
            wo_sb = []
            for k in range(KT_U):
                t = w_p.tile([P, OUTP], BF16, name=f"wo_{k}")
                sdma(out=t[:], in_=Wo_d[k])
                wo_sb.append(t)

            # ---- h^T (layer-0 GEMM stationary), bf16
            hT_sb = []
            for k in range(KT_IN):
                t = x_p.tile([P, NLOC], BF16, tag="x", name=f"ht_{k}")
                sdma(out=t[:], in_=hT_d[k])
                hT_sb.append(t)

            # ---- A^T resident in SBUF, fp8, slice-major pair layout
            at_sb = []
            off = 0
            for s, (_, w) in enumerate(SLICES):
                ncols = NPAIR * 2 * w
                t = at_p.tile([P, NPAIR, 2, w], FP8, name=f"at_s{s}")
                sdma(out=t[:], in_=ATs_d[:, off:off + ncols])
                at_sb.append(t)
                off += ncols

            # ---- JK running max (bf16)
            jk = [jk_p.tile([P, NLOC], BF16, name=f"jk_{ft}") for ft in range(KT_U)]

            # ---- collective buffers per (conv, group)
            loc_d, full_d = {}, {}
            for l in range(L + 1):
                width = U if l < L else OUTP
                for g in range(3):
                    loc_d[(l, g)] = dram_p.tile(
                        [P, GN[g] * width], FP8, name=f"loc_{l}_{g}")
                    full_d[(l, g)] = dram_p.tile(
                        [C, P, GN[g] * width], FP8, name=f"full_{l}_{g}",
                        addr_space="Shared")

            rg = [list(range(C))]

            def gemm_group(l, g, xt_tiles, w_tiles):
                """GEMM node-group g of conv l's messages, quantize to fp8,
                write to DRAM, AllGather.  l==L is the final conv (from jk)."""
                nt0, nn = GROUPS[g]
                width = U if l < L else OUTP
                stage = stg_p.tile([P, 4 * U], FP8, tag="stg",
                                   name=f"stg_{l}_{g}")
                for i in range(nn):
                    nt = nt0 + i
                    ps = psmm_p.tile([P, U], F32, tag="mm", name=f"mm_{l}_{g}_{i}")
                    kt = len(xt_tiles)
                    for k in range(kt):
                        nc.tensor.matmul(
                            ps[:, :width],
                            lhsT=xt_tiles[k][:, nt * P:(nt + 1) * P],
                            rhs=w_tiles[k][:, :width],
                            start=(k == 0), stop=(k == kt - 1),
                        )
                    if l < L:
                        qs = scales[:, l:l + 1]
                    else:
                        qs = fincons[:, nt:nt + 1]  # dinv * s_fin (per node)
                    nc.vector.tensor_scalar_mul(
                        stage[:, i * width:(i + 1) * width], ps[:, :width], qs)
                    if l == L:
                        # self-loop term for the final conv: m/deg + bo
                        t1 = sm_p.tile([P, OUT_REAL], F32, tag="sm",
                                       name=f"st1_{nt}")
                        nc.vector.tensor_scalar_mul(
                            t1[:], ps[:, :OUT_REAL], fincons[:, 2 * NT + nt:2 * NT + nt + 1])
                        t2 = sb_p.tile([P, OUT_REAL], F32, tag="selfbo",
                                       name=f"selfbo_{nt}")
                        nc.vector.tensor_tensor(
                            out=t2[:], in0=t1[:], in1=bob[:], op=ALU.add)
                        selfbo[nt] = t2
                adma(out=loc_d[(l, g)][:], in_=stage[:, :nn * width])
                nc.gpsimd.collective_compute(
                    "AllGather", ALU.bypass, replica_groups=rg,
                    ins=[loc_d[(l, g)].opt()],
                    outs=[full_d[(l, g)].opt()],
                )

            def load_m(l):
                """Load the 3 gathered message groups of conv l into SBUF.
                Split each load across both HWDGE rings so the two halves
                transfer in parallel."""
                width = U if l < L else OUTP
                tiles = []
                for g in range(3):
                    nn = GN[g]
                    if l == L:
                        t = mfin_p.tile([P, C, nn, OUTP], FP8, tag=f"mf{g}",
                                        name=f"mfin_{g}")
                    elif g < 2:
                        t = mg01_p.tile([P, C, nn, U], FP8, tag=f"m{g}",
                                        name=f"m_{l}_{g}")
                    else:
                        t = mg2_p.tile([P, C, nn, U], FP8, tag="m2",
                                       name=f"m_{l}_{g}")
                    src = full_d[(l, g)][:].rearrange("c p x -> p c x")
                    h = C // 2
                    sdma(out=t[:, 0:h], in_=src[:, 0:h])
                    adma(out=t[:, h:C], in_=src[:, h:C])
                    tiles.append(t)
                return tiles

            selfbo = [None] * NT

            # ================= conv 0 messages =================
            for g in range(3):
                gemm_group(0, g, hT_sb, w0_sb)

            # ================= conv layers =================
            xt = None
            for l in range(L):
                m_tiles = load_m(l)
                xt_new = [
                    x_p.tile([P, NLOC], BF16, tag="x", name=f"x{l + 1}_{ft}")
                    for ft in range(KT_U)
                ]
                for s, (off, w) in enumerate(SLICES):
                    pss = [
                        psagg_p.tile([P, 512], F32, tag="agg",
                                     name=f"agg_{l}_{s}_{ft}")
                        for ft in range(KT_U)
                    ]
                    for j, (g, c, pp) in enumerate(PAIRS):
                        rhs = at_sb[s][:, j]
                        for ft in range(KT_U):
                            nc.tensor.matmul(
                                pss[ft][:, :w],
                                lhsT=m_tiles[g][:, c, pp * 2:pp * 2 + 2,
                                                ft * P:(ft + 1) * P],
                                rhs=rhs,
                                start=(j == 0), stop=(j == NPAIR - 1),
                                perf_mode=DR,
                            )
                    # evacuate slice: x = relu(psum/s_l + b), jk = max(jk, x)
                    for ft in range(KT_U):
                        nc.scalar.activation(
                            xt_new[ft][:, off:off + w], pss[ft][:, :w], AF.Relu,
                            bias=biases[:, 2 * l + ft:2 * l + ft + 1],
                            scale=scales[:, L + l:L + l + 1],
                        )
                        if l == 0:
                            nc.vector.tensor_copy(
                                out=jk[ft][:, off:off + w],
                                in_=xt_new[ft][:, off:off + w])
                        else:
                            nc.vector.tensor_tensor(
                                out=jk[ft][:, off:off + w],
                                in0=jk[ft][:, off:off + w],
                                in1=xt_new[ft][:, off:off + w], op=ALU.max)
                    # produce next conv's messages for this node group + AG
                    if l < L - 1:
                        gemm_group(l + 1, s, xt_new, wh_sb[l])
                    else:
                        gemm_group(L, s, jk, wo_sb)
                xt = xt_new

            # ================= final conv aggregation =================
            m_tiles = load_m(L)
            aggF = af_p.tile([P, NLOC], F32, name="aggF")
            for s, (off, w) in enumerate(SLICES):
                ps = psagg_p.tile([P, 512], F32, tag="agg", name=f"aggf_{s}")
                for j, (g, c, pp) in enumerate(PAIRS):
                    nc.tensor.matmul(
                        ps[:, :w],
                        lhsT=m_tiles[g][:, c, pp * 2:pp * 2 + 2, 0:OUTP],
                        rhs=at_sb[s][:, j],
                        start=(j == 0), stop=(j == NPAIR - 1),
                        perf_mode=DR,
                    )
                nc.vector.tensor_copy(out=aggF[:, off:off + w], in_=ps[:, :w])

            # ================= normalize + self loop + log_softmax ==========
            # stage-major batching: engines pipeline across node tiles and
            # the scalar engine loads each activation table exactly once
            z3s, lsums = [], []
            for nt in range(NT):
                ps_t = pstr_p.tile([P, P], F32, tag="tr", name=f"tr_{nt}")
                nc.tensor.transpose(
                    out=ps_t[:], in_=aggF[:, nt * P:(nt + 1) * P],
                    identity=ident[:])
                z = sm_p.tile([P, OUT_REAL], F32, tag="sm", name=f"z_{nt}")
                nc.vector.tensor_scalar_mul(
                    z[:], ps_t[:, :OUT_REAL], fincons[:, NT + nt:NT + nt + 1])
                z2 = sm_p.tile([P, OUT_REAL], F32, tag="sm", name=f"z2_{nt}")
                nc.vector.tensor_tensor(
                    out=z2[:], in0=z[:], in1=selfbo[nt][:], op=ALU.add)
                rmax = sm_p.tile([P, 1], F32, tag="r1", name=f"rmax_{nt}")
                nc.vector.reduce_max(rmax[:], z2[:], axis=AX.X)
                z3 = sb_p.tile([P, OUT_REAL], F32, tag="z3", name=f"z3_{nt}")
                nc.vector.tensor_scalar_sub(z3[:], z2[:], rmax[:])
                z3s.append(z3)
            ezs = []
            for nt in range(NT):
                ez = sm_p.tile([P, OUT_REAL], F32, tag="sm", name=f"ez_{nt}")
                nc.scalar.activation(ez[:], z3s[nt][:], AF.Exp)
                ezs.append(ez)
            ssums = []
            for nt in range(NT):
                ssum = sm_p.tile([P, 1], F32, tag="r1", name=f"ssum_{nt}")
                nc.vector.reduce_sum(ssum[:], ezs[nt][:], axis=AX.X)
                ssums.append(ssum)
            for nt in range(NT):
                lsum = sm_p.tile([P, 1], F32, tag="r1", name=f"lsum_{nt}")
                nc.scalar.activation(lsum[:], ssums[nt][:], AF.Ln)
                lsums.append(lsum)
            for nt in range(NT):
                o = sm_p.tile([P, OUT_REAL], F32, tag="sm", name=f"o_{nt}")
                nc.vector.tensor_scalar_sub(o[:], z3s[nt][:], lsums[nt][:])
                adma(out=out_d[nt * P:(nt + 1) * P, :], in_=o[:])

    nc.compile()
    return nc


# ---------------------------------------------------------------- host prep
def _forward_scales(h, edge_index, W0, b0, Wh, bh, Wo, bo, deg, dinv):
    """Cheap fp32 forward (sparse) to get per-layer max|m| for fp8 scaling."""
    import scipy.sparse as sp
    src = np.asarray(edge_index[0], np.int64)
    dst = np.asarray(edge_index[1], np.int64)
    A = sp.csr_matrix(
        (np.ones(len(src), np.float32), (dst, src)), shape=(N_NODES, N_NODES))
    x = np.asarray(h, np.float32)
    smax = []
    outs = []
    for l in range(L):
        W = np.asarray(W0 if l == 0 else Wh[l - 1], np.float32)
        b = np.asarray(b0 if l == 0 else bh[l - 1], np.float32)
        m = x @ W
        smax.append(np.abs(m).max())
        x = np.maximum(A @ m + b, 0.0)
        outs.append(x)
    xj = np.max(np.stack(outs), 0)
    mo = xj @ np.asarray(Wo, np.float32)
    smax.append(np.abs(mo * dinv[:, None]).max())
    return smax


def host_prep(h, edge_index, W0, b0, Wh, bh, Wo, bo):
    bf = ml_dtypes.bfloat16
    src = np.asarray(edge_index[0], np.int64)
    dst = np.asarray(edge_index[1], np.int64)

    deg = np.zeros(N_NODES, np.float32)
    np.add.at(deg, dst, 1.0)
    deg += 1.0
    dinv = (deg ** -0.5).astype(np.float32)

    smax = _forward_scales(h, edge_index, W0, b0, Wh, bh, Wo, bo, deg, dinv)
    s_hid = [QMAX / max(v, 1e-30) for v in smax[:L]]
    s_fin = QMAX / max(smax[L], 1e-30)

    # padded global src index: core r, local i -> r*NLOC + i
    psrc = (src // NLOC_REAL) * NLOC + (src % NLOC_REAL)

    # shared (node-independent) tensors
    W0_a = np.asarray(W0, np.float32).astype(bf).reshape(KT_IN, P, U)
    Wh_a = np.asarray(Wh, np.float32).astype(bf).reshape(L - 1, KT_U, P, U)
    Wo_pad = np.zeros((U, OUTP), np.float32)
    Wo_pad[:, :OUT_REAL] = np.asarray(Wo, np.float32)
    Wo_a = Wo_pad.astype(bf).reshape(KT_U, P, OUTP)
    biases = np.zeros((P, 2 * L), np.float32)
    for l in range(L):
        b = np.asarray(b0 if l == 0 else bh[l - 1], np.float32)
        for ft in range(KT_U):
            biases[:, 2 * l + ft] = b[ft * P:(ft + 1) * P]
    scales = np.zeros((P, 2 * L), np.float32)
    for l in range(L):
        scales[:, l] = s_hid[l]
        scales[:, L + l] = 1.0 / s_hid[l]
    bob = np.broadcast_to(
        np.asarray(bo, np.float32)[None, :OUT_REAL], (P, OUT_REAL)).copy()

    in_maps = []
    for c in range(C):
        lo, hi = c * NLOC_REAL, min((c + 1) * NLOC_REAL, N_NODES)
        sel = (dst >= lo) & (dst < hi)
        s_c = psrc[sel]
        d_c = dst[sel] - lo

        cnt = np.bincount(s_c * NLOC + d_c, minlength=NFULL * NLOC)
        A3 = cnt.astype(np.float32).reshape(KT, P, NLOC)
        assert cnt.max() <= 16, "edge multiplicity too large for exact fp8"

        blocks = []
        for s, (off, w) in enumerate(SLICES):
            for (g, cc, pp) in PAIRS:
                nt0 = GROUPS[g][0] + 2 * pp
                t0 = cc * NT + nt0
                blocks.append(A3[t0, :, off:off + w])
                blocks.append(A3[t0 + 1, :, off:off + w])
        ATs = np.concatenate(blocks, axis=1).astype(F8NP)

        hT = np.zeros((IN_FEATS, NLOC), np.float32)
        hT[:, :hi - lo] = np.asarray(h[lo:hi], np.float32).T
        hT = hT.astype(bf).reshape(KT_IN, P, NLOC)

        dinv_l = np.ones(NLOC, np.float32)
        deg_l = np.ones(NLOC, np.float32)
        dinv_l[:hi - lo] = dinv[lo:hi]
        deg_l[:hi - lo] = deg[lo:hi]
        fincons = np.zeros((P, 3 * NT), np.float32)
        for nt in range(NT):
            sl = slice(nt * P, (nt + 1) * P)
            fincons[:, nt] = dinv_l[sl] * s_fin
            fincons[:, NT + nt] = dinv_l[sl] / s_fin
            fincons[:, 2 * NT + nt] = 1.0 / deg_l[sl]

        in_maps.append({
            "ATs": ATs,
            "hT": hT.copy(),
            "W0": W0_a.copy(),
            "Wh": Wh_a.copy(),
            "Wo": Wo_a.copy(),
            "biases": biases.copy(),
            "scales": scales.copy(),
            "fincons": fincons,
            "bob": bob.copy(),
        })
    return in_maps


_CACHE = {}


def _get_nc():
    if "nc" not in _CACHE:
        _CACHE["nc"] = build_nc()
    return _CACHE["nc"]


def kernel(h, edge_index, W0, b0, Wh, bh, Wo, bo, _trace=False, _trace_kwargs=None):
    nc = _get_nc()
    in_maps = host_prep(h, edge_index, W0, b0, Wh, bh, Wo, bo)
    res = run_bass_kernel_spmd(
        nc, in_maps, list(range(C)),
        trace=_trace, **(_trace_kwargs or {}),
    )
    outs = [np.asarray(res.results[c]["out"])[:NLOC_REAL] for c in range(C)]
    full = np.concatenate(outs, axis=0)[:N_NODES].astype(np.float32)
    if _trace:
        return full, res
    return full
